# revision 1
# baseline (speedup 1.0000x reference)
"""Trainium2 Bass kernel for CrossModalMultiHeadAttentionK.

Computation (see reference): per-channel 7x7 local attention on a 40x40 grid,
B=2, C=256, with 1x1 convs (q/k/v/out/fuse) and sinusoidal positional
encodings. Sharding: 8 cores = (batch b in {0,1}) x (row-quarter q in {0..3},
10 output rows each). Each core holds all 256 channels in SBUF layout
[128 partitions, 2 channel-slots, spatial] so elementwise attention ops run
with free-dim 800 and no cross-core collectives are needed.

Engine plan per core:
 - PE (fp32): q/k/v 1x1 convs (pe-const folded in as extra accumulation
   matmuls), vo conv, fuse conv; plus fp16 identity-matmuls accumulating
   softmax numerator/denominator over the 49 window offsets into PSUM.
 - DVE (fp16 2x mode): s_j = q*k_j and p_j = e_j*v_j tensor_tensor muls.
   k/v have +1-element-shifted fp16 copies so odd window offsets stay
   4B-aligned (2x_1p requirement).
 - ACT: exp (table-based), PSUM evictions with per-channel bias.
"""

import math
import numpy as np

# ---- problem constants (hardcoded per harness contract) ----
B, C, H, W = 2, 256, 40, 40
KS, PAD = 7, 3
HEAD_DIM = 32
SCALING = HEAD_DIM ** -0.5
TEMPERATURE, PESCALE, EPS = 10000.0, 2.0 * math.pi, 1e-6
NQ = 4                 # row-quarters
RQ = H // NQ           # 10 output rows per core
NPOS = RQ * W          # 400 output positions per slot
KROWS = RQ + KS - 1    # 16 padded rows needed
KW = W + 2 * PAD       # 46 padded cols
KFREE = KROWS * KW     # 736
NF = 800               # 2 slots * NPOS, elementwise free dim
NJ = KS * KS           # 49 window offsets

_CACHE = {}


def _sine_pe(mask):
    """numpy port of reference.sine_pe; mask (b,h,w) bool."""
    nm = (~mask).astype(np.float32)
    y = np.cumsum(nm, axis=1, dtype=np.float32)
    x = np.cumsum(nm, axis=2, dtype=np.float32)
    y = y / (y[:, -1:, :] + EPS) * PESCALE
    x = x / (x[:, :, -1:] + EPS) * PESCALE
    nf = C // 2
    i = np.arange(nf, dtype=np.float32)
    dim_t = (TEMPERATURE ** (2.0 * np.floor(i / 2.0) / nf)).astype(np.float32)
    px = (x[..., None] / dim_t).astype(np.float32)
    py = (y[..., None] / dim_t).astype(np.float32)

    def interleave(p):
        s = np.stack([np.sin(p[..., 0::2]), np.cos(p[..., 1::2])], axis=4)
        return s.reshape(p.shape[0], p.shape[1], p.shape[2], -1)

    pos = np.concatenate([interleave(py), interleave(px)], axis=3)
    return pos.transpose(0, 3, 1, 2).astype(np.float32)  # (b, C, h, w)


def _pe_constants():
    if "pe" in _CACHE:
        return _CACHE["pe"]
    mask_q = np.zeros((1, H, W), dtype=bool)
    pe_q = _sine_pe(mask_q)[0]  # (C, H, W)
    Hp, Wp = H + 2 * PAD, W + 2 * PAD
    mask_k = np.zeros((1, Hp, Wp), dtype=bool)
    mask_k[:, :PAD, :] = True
    mask_k[:, :, :PAD] = True
    mask_k[:, Hp - PAD:, :] = True
    mask_k[:, :, Wp - PAD:] = True
    pe_k = _sine_pe(mask_k)[0]  # (C, Hp, Wp)
    _CACHE["pe"] = (pe_q, pe_k)
    return pe_q, pe_k


def _build_module():
    """Build (once) the per-core Bacc module. Same NEFF on all 8 cores."""
    if "nc" in _CACHE:
        return _CACHE["nc"]
    import concourse.bacc as bacc
    import concourse.tile as tile
    import concourse.mybir as mybir

    f32 = mybir.dt.float32
    f16 = mybir.dt.float16
    AF = mybir.ActivationFunctionType

    nc = bacc.Bacc("TRN2", target_bir_lowering=False, debug=False,
                   enable_asserts=True, num_devices=8)

    din = {}
    for name, shape, dt in [
        ("query", [128, 2, NPOS], f32),
        ("keypad", [128, 2, KFREE], f32),
        ("peq", [128, 2, NPOS], f16),
        ("pek", [128, 2, KFREE], f16),
        ("cf", [128, 2, NPOS], f32),
        ("wq", [2, 128, 256], f16),
        ("wk", [2, 128, 256], f16),
        ("wv", [2, 128, 256], f16),
        ("wo", [2, 128, 256], f32),
        ("wf", [4, 128, 256], f32),
        ("bq", [128, 2], f32),
        ("bk", [128, 2], f32),
        ("bv", [128, 2], f32),
        ("bo", [128, 2], f32),
        ("ident", [128, 128], f16),
    ]:
        din[name] = nc.dram_tensor(name, shape, dt, kind="ExternalInput").ap()
    d_out = nc.dram_tensor("out_part", [128, 2, NPOS], f32, kind="ExternalOutput").ap()
    d_vo = nc.dram_tensor("vo_part", [128, 2, NPOS], f32, kind="ExternalOutput").ap()

    with tile.TileContext(nc) as tc:
        with tc.tile_pool(name="consts", bufs=1) as cp, \
             tc.tile_pool(name="work", bufs=1) as wp, \
             tc.tile_pool(name="sje", bufs=5) as sp, \
             tc.tile_pool(name="psacc", bufs=1, space="PSUM") as pa, \
             tc.tile_pool(name="psconv", bufs=2, space="PSUM") as pc:

            # ---- load inputs ----
            sb = {}
            # spread big input DMAs over several DGE queues so they run in
            # parallel instead of serializing on the Sync queue
            dma_engs = [nc.sync, nc.gpsimd, nc.scalar]
            _di = [0]

            def dma_in(out, in_):
                dma_engs[_di[0] % len(dma_engs)].dma_start(out=out, in_=in_)
                _di[0] += 1

            for name, shape, dt in [
                ("query", [128, 2, NPOS], f32),
                ("keypad", [128, 2, KFREE], f32),
                ("peq", [128, 2, NPOS], f16),
                ("pek", [128, 2, KFREE], f16),
                ("cf", [128, 2, NPOS], f32),
                ("bq", [128, 2], f32),
                ("bk", [128, 2], f32),
                ("bv", [128, 2], f32),
                ("bo", [128, 2], f32),
                ("ident", [128, 128], f16),
            ]:
                t = cp.tile(shape, dt, tag=name)
                dma_in(t[:], din[name][:])
                sb[name] = t
            for name, nk, wdt in [("wq", 2, f16), ("wk", 2, f16), ("wv", 2, f16),
                                  ("wo", 2, f32), ("wf", 4, f32)]:
                tiles = []
                for k in range(nk):
                    t = cp.tile([128, 256], wdt, tag=f"{name}{k}")
                    dma_in(t[:], din[name][k])
                    tiles.append(t)
                sb[name] = tiles

            # fp16 casts of conv moving operands (DVE is idle in preamble)
            query16 = wp.tile([128, 2, NPOS], f16, tag="query16")
            nc.vector.tensor_copy(query16[:], sb["query"][:])
            keypad16 = wp.tile([128, 2, KFREE], f16, tag="keypad16")
            nc.vector.tensor_copy(keypad16[:], sb["keypad"][:])

            # ---- q/k/v convs (pe folded in as extra matmuls) ----
            q_b = wp.tile([128, NF], f16, tag="q_b")
            k_b = wp.tile([128, 2 * KFREE], f16, tag="k_b")
            k_b1 = wp.tile([128, 2 * KFREE], f16, tag="k_b1")
            v_b = wp.tile([128, 2 * KFREE], f16, tag="v_b")
            v_b1 = wp.tile([128, 2 * KFREE], f16, tag="v_b1")

            # q conv: out fp16, scaled weights/bias, pe folded
            for o in range(2):
                ps = pc.tile([128, NPOS], f32, tag="convps")
                for k in range(2):
                    nc.tensor.matmul(ps[:], sb["wq"][k][:, o * 128:(o + 1) * 128],
                                     query16[:, k, :], start=(k == 0), stop=False)
                for k in range(2):
                    nc.tensor.matmul(ps[:], sb["wq"][k][:, o * 128:(o + 1) * 128],
                                     sb["peq"][:, k, :], start=False, stop=(k == 1))
                nc.scalar.activation(out=q_b[:, o * NPOS:(o + 1) * NPOS], in_=ps[:],
                                     func=AF.Identity, bias=sb["bq"][:, o:o + 1])

            # k conv (with pe) and v conv (no pe): 736 free -> 2 chunks of 368
            for name, wname, bias, dest, dest1, with_pe in [
                ("k", "wk", "bk", k_b, k_b1, True),
                ("v", "wv", "bv", v_b, v_b1, False),
            ]:
                for o in range(2):
                    ps = pc.tile([128, KFREE], f32, tag="convps")
                    # psum chunks must not straddle the 2KB bank boundary
                    for sl in (slice(0, 512), slice(512, KFREE)):
                        nmm = 4 if with_pe else 2
                        i = 0
                        for k in range(2):
                            nc.tensor.matmul(ps[:, sl],
                                             sb[wname][k][:, o * 128:(o + 1) * 128],
                                             keypad16[:, k, sl],
                                             start=(i == 0), stop=(i == nmm - 1))
                            i += 1
                        if with_pe:
                            for k in range(2):
                                nc.tensor.matmul(ps[:, sl],
                                                 sb[wname][k][:, o * 128:(o + 1) * 128],
                                                 sb["pek"][:, k, sl],
                                                 start=False, stop=(i == nmm - 1))
                                i += 1
                    nc.scalar.activation(out=dest[:, o * KFREE:(o + 1) * KFREE],
                                         in_=ps[:], func=AF.Identity,
                                         bias=sb[bias][:, o:o + 1])
                # shifted-by-one fp16 copy for odd window offsets (DVE)
                nc.vector.tensor_copy(dest1[:, 0:2 * KFREE - 1], dest[:, 1:2 * KFREE])

            # ---- attention j-loop ----
            # one PSUM tile per (num/den, half) so each matmul output sits in
            # a single bank
            num_ps = [pa.tile([128, NPOS], f32, tag=f"num{h}", name=f"num{h}")
                      for h in range(2)]
            den_ps = [pa.tile([128, NPOS], f32, tag=f"den{h}", name=f"den{h}")
                      for h in range(2)]
            q4 = q_b[:].rearrange("p (a r c) -> p a r c", a=2, r=RQ)
            k4 = k_b[:].rearrange("p (a r c) -> p a r c", a=2, r=KROWS)
            k41 = k_b1[:].rearrange("p (a r c) -> p a r c", a=2, r=KROWS)
            v4 = v_b[:].rearrange("p (a r c) -> p a r c", a=2, r=KROWS)
            v41 = v_b1[:].rearrange("p (a r c) -> p a r c", a=2, r=KROWS)

            for j in range(NJ):
                di, dj = j // KS, j % KS
                if dj % 2 == 0:
                    kv, vv, c0 = k4, v4, dj
                else:
                    kv, vv, c0 = k41, v41, dj - 1
                s_t = sp.tile([128, NF], f16, tag="s")
                s4 = s_t[:].rearrange("p (a r c) -> p a r c", a=2, r=RQ)
                # route a fraction of the qk muls to the otherwise-idle GPSIMD
                s_eng = nc.vector
                s_eng.tensor_mul(s4, q4, kv[:, :, di:di + RQ, c0:c0 + W])
                e_t = sp.tile([128, NF], f16, tag="e")
                nc.scalar.activation(out=e_t[:], in_=s_t[:], func=AF.Exp)
                p_t = sp.tile([128, NF], f16, tag="pp")
                p4 = p_t[:].rearrange("p (a r c) -> p a r c", a=2, r=RQ)
                nc.vector.tensor_mul(p4, e_t[:].rearrange("p (a r c) -> p a r c", a=2, r=RQ),
                                     vv[:, :, di:di + RQ, c0:c0 + W])
                for hh in range(2):
                    sl = slice(hh * NPOS, (hh + 1) * NPOS)
                    nc.tensor.matmul(num_ps[hh][:], sb["ident"][:], p_t[:, sl],
                                     start=(j == 0), stop=(j == NJ - 1))
                    nc.tensor.matmul(den_ps[hh][:], sb["ident"][:], e_t[:, sl],
                                     start=(j == 0), stop=(j == NJ - 1))

            # ---- normalize + vo conv + fuse conv, pipelined by spatial half ----
            HC = NPOS // 2  # 200-position chunks
            r_t = wp.tile([128, NF], f32, tag="r")
            att = wp.tile([128, NF], f32, tag="att")
            vo_sb = wp.tile([128, NF], f32, tag="vo")
            out_sb = wp.tile([128, NF], f32, tag="out")
            for cch in range(2):
                cs = slice(cch * HC, (cch + 1) * HC)
                for hh in range(2):
                    sl = slice(hh * NPOS + cch * HC, hh * NPOS + (cch + 1) * HC)
                    nc.vector.reciprocal(r_t[:, sl], den_ps[hh][:, cs])
                    nc.vector.tensor_mul(att[:, sl], num_ps[hh][:, cs], r_t[:, sl])
                for o in range(2):
                    ps = pc.tile([128, HC], f32, tag="convps", name="tailps")
                    for k in range(2):
                        nc.tensor.matmul(ps[:], sb["wo"][k][:, o * 128:(o + 1) * 128],
                                         att[:, k * NPOS + cch * HC:
                                             k * NPOS + (cch + 1) * HC],
                                         start=(k == 0), stop=(k == 1))
                    nc.scalar.activation(
                        out=vo_sb[:, o * NPOS + cch * HC:o * NPOS + (cch + 1) * HC],
                        in_=ps[:], func=AF.Identity, bias=sb["bo"][:, o:o + 1])
                for o in range(2):
                    ps = pc.tile([128, HC], f32, tag="convps", name="tailps")
                    i = 0
                    for k in range(2):
                        nc.tensor.matmul(ps[:], sb["wf"][k][:, o * 128:(o + 1) * 128],
                                         sb["query"][:, k, cs],
                                         start=(i == 0), stop=False)
                        i += 1
                    for k in range(2):
                        nc.tensor.matmul(ps[:], sb["wf"][2 + k][:, o * 128:(o + 1) * 128],
                                         vo_sb[:, k * NPOS + cch * HC:
                                               k * NPOS + (cch + 1) * HC],
                                         start=False, stop=(i == 3))
                        i += 1
                    # fuse pe contribution folded in host-side (cf)
                    nc.vector.tensor_add(
                        out_sb[:, o * NPOS + cch * HC:o * NPOS + (cch + 1) * HC],
                        ps[:], sb["cf"][:, o, cs])
                nc.sync.dma_start(out=d_vo[:, :, cs], in_=vo_sb[:].rearrange(
                    "p (a n) -> p a n", a=2)[:, :, cs])
                nc.sync.dma_start(out=d_out[:, :, cs], in_=out_sb[:].rearrange(
                    "p (a n) -> p a n", a=2)[:, :, cs])

    nc.compile()
    _CACHE["nc"] = nc
    return nc


def _in_maps(key, query, Wq, bq, Wk, bk, Wv, bv, Wo, bo, Wf):
    pe_q, pe_k = _pe_constants()
    keypad_full = np.pad(key, ((0, 0), (0, 0), (PAD, PAD), (PAD, PAD)))
    wqT = np.ascontiguousarray((Wq.T * SCALING).reshape(2, 128, 256)).astype(np.float16)
    wkT = np.ascontiguousarray(Wk.T.reshape(2, 128, 256)).astype(np.float16)
    wvT = np.ascontiguousarray(Wv.T.reshape(2, 128, 256)).astype(np.float16)
    woT = np.ascontiguousarray(Wo.T.reshape(2, 128, 256)).astype(np.float32)
    wfT = np.ascontiguousarray(Wf.T.reshape(4, 128, 256)).astype(np.float32)
    # fuse-conv pe contribution, folded host-side: Cf = Wf[:, :C] @ pe_q
    cf_full = np.einsum("oc,chw->ohw", Wf[:, :C].astype(np.float32),
                        pe_q).astype(np.float32)  # (C, H, W)
    bq_s = np.ascontiguousarray((bq * SCALING).reshape(2, 128).T).astype(np.float32)
    bk_s = np.ascontiguousarray(bk.reshape(2, 128).T).astype(np.float32)
    bv_s = np.ascontiguousarray(bv.reshape(2, 128).T).astype(np.float32)
    bo_s = np.ascontiguousarray(bo.reshape(2, 128).T).astype(np.float32)
    ident = np.eye(128, dtype=np.float16)

    def part(arr_cxn, npos):  # (C, rows, cols) -> (128, 2, rows*cols)
        return np.ascontiguousarray(
            arr_cxn.reshape(2, 128, npos).transpose(1, 0, 2)).astype(np.float32)

    maps = []
    for b in range(B):
        for q in range(NQ):
            r0 = RQ * q
            m = {
                "query": part(query[b, :, r0:r0 + RQ, :].reshape(C, NPOS), NPOS),
                "keypad": part(keypad_full[b, :, r0:r0 + KROWS, :].reshape(C, KFREE), KFREE),
                "peq": part(pe_q[:, r0:r0 + RQ, :].reshape(C, NPOS), NPOS).astype(np.float16),
                "pek": part(pe_k[:, r0:r0 + KROWS, :].reshape(C, KFREE), KFREE).astype(np.float16),
                "cf": part(cf_full[:, r0:r0 + RQ, :].reshape(C, NPOS), NPOS),
                "wq": wqT, "wk": wkT, "wv": wvT, "wo": woT, "wf": wfT,
                "bq": bq_s, "bk": bk_s, "bv": bv_s, "bo": bo_s,
                "ident": ident,
            }
            maps.append(m)
    return maps


def kernel(key, query, Wq, bq, Wk, bk, Wv, bv, Wo, bo, Wf, _trace=False):
    from concourse.bass_utils import run_bass_kernel_spmd

    args = [np.asarray(a, dtype=np.float32) for a in
            (key, query, Wq, bq, Wk, bk, Wv, bv, Wo, bo, Wf)]
    nc = _build_module()
    maps = _in_maps(*args)
    res = run_bass_kernel_spmd(nc, maps, list(range(8)), trace=_trace)
    _CACHE["last_res"] = res

    out = np.zeros((B, C, H, W), dtype=np.float32)
    vo = np.zeros((B, C, H, W), dtype=np.float32)
    for b in range(B):
        for q in range(NQ):
            r = res.results[b * NQ + q]
            r0 = RQ * q
            out[b, :, r0:r0 + RQ, :] = r["out_part"].transpose(1, 0, 2).reshape(C, RQ, W)
            vo[b, :, r0:r0 + RQ, :] = r["vo_part"].transpose(1, 0, 2).reshape(C, RQ, W)
    return out, vo



# revision 8
# speedup vs baseline: 1.1180x; 1.1180x over previous
"""Trainium2 Bass kernel for CrossModalMultiHeadAttentionK.

Per-channel 7x7 local attention on a 40x40 grid, B=2, C=256, with 1x1 convs
(q/k/v/out/fuse) and sinusoidal positional encodings. Sharding: 8 cores =
(batch b in {0,1}) x (row-quarter q in {0..3}, 10 output rows each). Each core
holds all 256 channels in SBUF layout [128 partitions, 2 channel-slots,
spatial].

v2 restructure vs baseline:
 - host-side: pe folded into query/key inputs, fp16 upload (no on-device
   casts), all weights fp16, biases concatenated into one tensor.
 - j-loop grouped per di row (7 outer steps instead of 49): DVE muls grouped
   over dj with overlapping strided APs (even dj from k_b, odd dj from the
   +1-shifted k_b1 so fp16 2x mode holds), ONE 5600-elem EXP per di, and
   num/den accumulated with stride-0-output identity matmuls (2800 cols per
   mm) so PE does 4 matmuls per di instead of 28.
 - tail: reciprocal via ACT ln + exp(-x) (same table set as Exp), fp16
   convs, fp16 outputs (host casts back to fp32).
"""

import math
import numpy as np

# ---- problem constants (hardcoded per harness contract) ----
B, C, H, W = 2, 256, 40, 40
KS, PAD = 7, 3
HEAD_DIM = 32
SCALING = HEAD_DIM ** -0.5
TEMPERATURE, PESCALE, EPS = 10000.0, 2.0 * math.pi, 1e-6
NQ = 4                 # row-quarters
RQ = H // NQ           # 10 output rows per core
NPOS = RQ * W          # 400 output positions per slot
KROWS = RQ + KS - 1    # 16 padded rows needed
KW = W + 2 * PAD       # 46 padded cols
KFREE = KROWS * KW     # 736
NF = 800               # 2 slots * NPOS
NJ = KS * KS           # 49 window offsets
NEV, NOD = 4, 3        # even/odd dj counts

_CACHE = {}


def _sine_pe(mask):
    """numpy port of reference.sine_pe; mask (b,h,w) bool."""
    nm = (~mask).astype(np.float32)
    y = np.cumsum(nm, axis=1, dtype=np.float32)
    x = np.cumsum(nm, axis=2, dtype=np.float32)
    y = y / (y[:, -1:, :] + EPS) * PESCALE
    x = x / (x[:, :, -1:] + EPS) * PESCALE
    nf = C // 2
    i = np.arange(nf, dtype=np.float32)
    dim_t = (TEMPERATURE ** (2.0 * np.floor(i / 2.0) / nf)).astype(np.float32)
    px = (x[..., None] / dim_t).astype(np.float32)
    py = (y[..., None] / dim_t).astype(np.float32)

    def interleave(p):
        s = np.stack([np.sin(p[..., 0::2]), np.cos(p[..., 1::2])], axis=4)
        return s.reshape(p.shape[0], p.shape[1], p.shape[2], -1)

    pos = np.concatenate([interleave(py), interleave(px)], axis=3)
    return pos.transpose(0, 3, 1, 2).astype(np.float32)  # (b, C, h, w)


def _pe_constants():
    if "pe" in _CACHE:
        return _CACHE["pe"]
    mask_q = np.zeros((1, H, W), dtype=bool)
    pe_q = _sine_pe(mask_q)[0]  # (C, H, W)
    Hp, Wp = H + 2 * PAD, W + 2 * PAD
    mask_k = np.zeros((1, Hp, Wp), dtype=bool)
    mask_k[:, :PAD, :] = True
    mask_k[:, :, :PAD] = True
    mask_k[:, Hp - PAD:, :] = True
    mask_k[:, :, Wp - PAD:] = True
    pe_k = _sine_pe(mask_k)[0]  # (C, Hp, Wp)
    _CACHE["pe"] = (pe_q, pe_k)
    return pe_q, pe_k


def _build_module():
    """Build (once) the per-core Bacc module. Same NEFF on all 8 cores."""
    if "nc" in _CACHE:
        return _CACHE["nc"]
    import concourse.bacc as bacc
    import concourse.tile as tile
    import concourse.mybir as mybir
    from concourse.ap import AP

    f32 = mybir.dt.float32
    f16 = mybir.dt.float16
    AF = mybir.ActivationFunctionType

    nc = bacc.Bacc("TRN2", target_bir_lowering=False, debug=False,
                   enable_asserts=True, num_devices=8)

    din = {}
    for name, shape, dt in [
        ("qeff", [128, 2, NPOS], f16),     # query + pe_q
        ("kpe", [128, 2, KFREE], f16),     # padded key + pe_k
        ("kraw", [128, 2, KFREE], f16),    # padded key (no pe), for v conv
        ("wq", [2, 128, 256], f16),        # pre-scaled by SCALING
        ("wk", [2, 128, 256], f16),
        ("wv", [2, 128, 256], f16),
        ("wo", [2, 128, 256], f16),
        ("wf", [4, 128, 256], f16),
        ("bias", [128, 4, 2], f32),        # [bq|bk|bv|bo] x [o0|o1]
        ("ident", [128, 128], f16),
    ]:
        din[name] = nc.dram_tensor(name, shape, dt, kind="ExternalInput").ap()
    d_out = nc.dram_tensor("out16", [128, 2, NPOS], f16, kind="ExternalOutput").ap()
    d_vo = nc.dram_tensor("vo16", [128, 2, NPOS], f16, kind="ExternalOutput").ap()

    with tile.TileContext(nc) as tc:
        with tc.tile_pool(name="consts", bufs=1) as cp, \
             tc.tile_pool(name="work", bufs=1) as wp, \
             tc.tile_pool(name="sje", bufs=4) as sp, \
             tc.tile_pool(name="psacc", bufs=1, space="PSUM") as pa, \
             tc.tile_pool(name="psconv", bufs=2, space="PSUM") as pc:

            # ---- load inputs: k-path on sync queue, q/v-path on scalar ----
            sb = {}

            def load(qeng, name):
                shape = list(din[name].shape)
                if shape[0] != 128:  # weights [k, 128, 256] -> per-k tiles
                    tiles = []
                    for k in range(shape[0]):
                        t = cp.tile(shape[1:], din[name].dtype, tag=f"{name}{k}")
                        qeng.dma_start(out=t[:], in_=din[name][k])
                        tiles.append(t)
                    sb[name] = tiles
                else:
                    t = cp.tile(shape, din[name].dtype, tag=name)
                    qeng.dma_start(out=t[:], in_=din[name][:])
                    sb[name] = t

            load(nc.sync, "wk")
            load(nc.sync, "kpe")
            load(nc.scalar, "wq")
            load(nc.scalar, "qeff")
            load(nc.sync, "ident")
            load(nc.sync, "bias")
            load(nc.scalar, "wv")
            load(nc.scalar, "kraw")
            load(nc.sync, "wo")
            load(nc.scalar, "wf")

            bias = sb["bias"]

            # ---- q/k/v convs (pe already folded into qeff/kpe) ----
            q_b = wp.tile([128, 2 * NPOS], f16, tag="q_b")
            k_b = wp.tile([128, 2 * KFREE], f16, tag="k_b")
            k_b1 = wp.tile([128, 2 * KFREE], f16, tag="k_b1")
            v_b = wp.tile([128, 2 * KFREE], f16, tag="v_b")
            v_b1 = wp.tile([128, 2 * KFREE], f16, tag="v_b1")

            def conv(wname, src, dest, dfree, bias_row):
                # dest[o*dfree : (o+1)*dfree] = W[:, o]ᵀ@src + b[o], fp16
                for o in range(2):
                    ps = pc.tile([128, KFREE], f32, tag="convps")
                    # psum chunks must not straddle the 2KB bank boundary
                    sls = [slice(0, dfree)] if dfree <= 512 else \
                        [slice(0, 512), slice(512, dfree)]
                    for sl in sls:
                        for k in range(2):
                            nc.tensor.matmul(
                                ps[:, sl],
                                sb[wname][k][:, o * 128:(o + 1) * 128],
                                src[:][:, k, sl],
                                start=(k == 0), stop=(k == 1))
                    nc.scalar.activation(
                        out=dest[:, o * dfree:(o + 1) * dfree],
                        in_=ps[:, 0:dfree], func=AF.Identity,
                        bias=bias[:][:, bias_row, o:o + 1])

            conv("wk", sb["kpe"], k_b, KFREE, 1)
            conv("wq", sb["qeff"], q_b, NPOS, 0)
            conv("wv", sb["kraw"], v_b, KFREE, 2)
            # +1-element-shifted fp16 copies so odd window offsets stay
            # 4B-aligned (DVE 2x_1p requirement)
            nc.vector.tensor_copy(k_b1[:, 0:2 * KFREE - 1], k_b[:, 1:2 * KFREE])
            nc.vector.tensor_copy(v_b1[:, 0:2 * KFREE - 1], v_b[:, 1:2 * KFREE])

            # ---- attention j-loop, grouped per di row ----
            num_ps = [pa.tile([128, NPOS], f32, tag=f"num{h}", name=f"num{h}")
                      for h in range(2)]
            den_ps = [pa.tile([128, NPOS], f32, tag=f"den{h}", name=f"den{h}")
                      for h in range(2)]

            # s/e/p layout per di: [128, (a, djslot, pos)] where djslot 0..3
            # = dj in {0,2,4,6}, djslot 4..6 = dj in {1,3,5}
            def blk(t, a, s0, nslot):
                # [p][djslot][r][c] view of slots s0..s0+nslot
                return AP(t[:].tensor, t[:].offset + a * KS * NPOS + s0 * NPOS,
                          [list(t[:].ap[0]), [NPOS, nslot], [W, RQ], [1, W]])

            def qblk(a, nslot):
                # q_b[:, a, :] broadcast over djslot
                base = q_b[:]
                return AP(base.tensor, base.offset + a * NPOS,
                          [list(base.ap[0]), [0, nslot], [W, RQ], [1, W]])

            def kvblk(t, a, di, dj0, nslot):
                # t[:, a, di:di+10, dj0 + 2*slot + c] overlapping window view
                base = t[:]
                return AP(base.tensor,
                          base.offset + a * KFREE + di * KW + dj0,
                          [list(base.ap[0]), [2, nslot], [KW, RQ], [1, W]])

            for di in range(KS):
                s_t = sp.tile([128, 2 * KS * NPOS], f16, tag="s")
                for a in range(2):
                    nc.vector.tensor_mul(blk(s_t, a, 0, NEV), qblk(a, NEV),
                                         kvblk(k_b, a, di, 0, NEV))
                    nc.vector.tensor_mul(blk(s_t, a, NEV, NOD), qblk(a, NOD),
                                         kvblk(k_b1, a, di, 0, NOD))
                e_t = sp.tile([128, 2 * KS * NPOS], f16, tag="e")
                nc.scalar.activation(out=e_t[:], in_=s_t[:], func=AF.Exp)
                p_t = sp.tile([128, 2 * KS * NPOS], f16, tag="pp")
                for a in range(2):
                    nc.vector.tensor_mul(blk(p_t, a, 0, NEV),
                                         blk(e_t, a, 0, NEV),
                                         kvblk(v_b, a, di, 0, NEV))
                    nc.vector.tensor_mul(blk(p_t, a, NEV, NOD),
                                         blk(e_t, a, NEV, NOD),
                                         kvblk(v_b1, a, di, 0, NOD))
                # ISA caps one matmul's out free-size at one PSUM bank, so
                # the dj sum is one 400-col identity matmul per slot
                for a in range(2):
                    for ps, t in ((den_ps[a], e_t), (num_ps[a], p_t)):
                        for dj in range(KS):
                            o0 = a * KS * NPOS + dj * NPOS
                            nc.tensor.matmul(
                                ps[:], sb["ident"][:], t[:][:, o0:o0 + NPOS],
                                start=(di == 0 and dj == 0),
                                stop=(di == KS - 1 and dj == KS - 1))

            # ---- tail: att = num * exp(-ln(den)), vo conv, fuse conv ----
            att = wp.tile([128, 2 * NPOS], f16, tag="att")
            vo_sb = wp.tile([128, 2 * NPOS], f16, tag="vo")
            out_sb = wp.tile([128, 2 * NPOS], f16, tag="out")
            ln_t = wp.tile([128, 2 * NPOS], f32, tag="ln")
            r_t = wp.tile([128, 2 * NPOS], f32, tag="r")
            for a in range(2):
                sl = slice(a * NPOS, (a + 1) * NPOS)
                nc.scalar.activation(out=ln_t[:, sl], in_=den_ps[a][:],
                                     func=AF.Ln)
                nc.scalar.activation(out=r_t[:, sl], in_=ln_t[:, sl],
                                     func=AF.Exp, scale=-1.0)
                nc.vector.tensor_mul(att[:, sl], num_ps[a][:], r_t[:, sl])
            for o in range(2):
                ps = pc.tile([128, NPOS], f32, tag="convps", name="vops")
                for k in range(2):
                    nc.tensor.matmul(ps[:],
                                     sb["wo"][k][:, o * 128:(o + 1) * 128],
                                     att[:, k * NPOS:(k + 1) * NPOS],
                                     start=(k == 0), stop=(k == 1))
                nc.scalar.activation(out=vo_sb[:, o * NPOS:(o + 1) * NPOS],
                                     in_=ps[:], func=AF.Identity,
                                     bias=bias[:][:, 3, o:o + 1])
            nc.sync.dma_start(
                out=d_vo[:], in_=vo_sb[:].rearrange("p (a n) -> p a n", a=2))
            for o in range(2):
                ps = pc.tile([128, NPOS], f32, tag="convps", name="fuseps")
                i = 0
                for k in range(2):
                    nc.tensor.matmul(ps[:],
                                     sb["wf"][k][:, o * 128:(o + 1) * 128],
                                     sb["qeff"][:][:, k, :],
                                     start=(i == 0), stop=False)
                    i += 1
                for k in range(2):
                    nc.tensor.matmul(ps[:],
                                     sb["wf"][2 + k][:, o * 128:(o + 1) * 128],
                                     vo_sb[:, k * NPOS:(k + 1) * NPOS],
                                     start=False, stop=(i == 3))
                    i += 1
                nc.scalar.activation(out=out_sb[:, o * NPOS:(o + 1) * NPOS],
                                     in_=ps[:], func=AF.Identity)
            nc.scalar.dma_start(
                out=d_out[:], in_=out_sb[:].rearrange("p (a n) -> p a n", a=2))

    nc.compile()
    _CACHE["nc"] = nc
    return nc


def _in_maps(key, query, Wq, bq, Wk, bk, Wv, bv, Wo, bo, Wf):
    pe_q, pe_k = _pe_constants()
    keypad = np.pad(key, ((0, 0), (0, 0), (PAD, PAD), (PAD, PAD)))
    qeff_full = query + pe_q[None]          # (B, C, H, W)
    kpe_full = keypad + pe_k[None]          # (B, C, 46, 46)
    wq16 = np.ascontiguousarray((Wq.T * SCALING).reshape(2, 128, 256)).astype(np.float16)
    wk16 = np.ascontiguousarray(Wk.T.reshape(2, 128, 256)).astype(np.float16)
    wv16 = np.ascontiguousarray(Wv.T.reshape(2, 128, 256)).astype(np.float16)
    wo16 = np.ascontiguousarray(Wo.T.reshape(2, 128, 256)).astype(np.float16)
    wf16 = np.ascontiguousarray(Wf.T.reshape(4, 128, 256)).astype(np.float16)
    # bias tensor [128, 4, 2]: rows bq*SCALING, bk, bv, bo
    bias = np.stack([(bq * SCALING).reshape(2, 128).T, bk.reshape(2, 128).T,
                     bv.reshape(2, 128).T, bo.reshape(2, 128).T],
                    axis=1).astype(np.float32)
    bias = np.ascontiguousarray(bias)
    ident = np.eye(128, dtype=np.float16)

    def part(arr, npos):  # (C, rows*cols) -> (128, 2, rows*cols) fp16
        return np.ascontiguousarray(
            arr.reshape(2, 128, npos).transpose(1, 0, 2)).astype(np.float16)

    maps = []
    for b in range(B):
        for q in range(NQ):
            r0 = RQ * q
            m = {
                "qeff": part(qeff_full[b, :, r0:r0 + RQ, :].reshape(C, NPOS), NPOS),
                "kpe": part(kpe_full[b, :, r0:r0 + KROWS, :].reshape(C, KFREE), KFREE),
                "kraw": part(keypad[b, :, r0:r0 + KROWS, :].reshape(C, KFREE), KFREE),
                "wq": wq16, "wk": wk16, "wv": wv16, "wo": wo16, "wf": wf16,
                "bias": bias, "ident": ident,
            }
            maps.append(m)
    return maps


def kernel(key, query, Wq, bq, Wk, bk, Wv, bv, Wo, bo, Wf, _trace=False):
    from concourse.bass_utils import run_bass_kernel_spmd

    args = [np.asarray(a, dtype=np.float32) for a in
            (key, query, Wq, bq, Wk, bk, Wv, bv, Wo, bo, Wf)]
    nc = _build_module()
    maps = _in_maps(*args)
    res = run_bass_kernel_spmd(nc, maps, list(range(8)), trace=_trace)
    _CACHE["last_res"] = res

    out = np.zeros((B, C, H, W), dtype=np.float32)
    vo = np.zeros((B, C, H, W), dtype=np.float32)
    for b in range(B):
        for q in range(NQ):
            r = res.results[b * NQ + q]
            r0 = RQ * q
            out[b, :, r0:r0 + RQ, :] = (
                r["out16"].astype(np.float32).transpose(1, 0, 2).reshape(C, RQ, W))
            vo[b, :, r0:r0 + RQ, :] = (
                r["vo16"].astype(np.float32).transpose(1, 0, 2).reshape(C, RQ, W))
    return out, vo
